# revision 2
# baseline (speedup 1.0000x reference)
"""GCLSTM Trainium2 Bass kernel.

Data-parallel over batch B=64 across 8 NeuronCores (8 batches/core).
Host (numpy) pre-slices per-core tensors, pre-transposes layouts, permutes
LSTM gate order to [i,f,o,g], doubles the g-gate weights (tanh(z) =
2*sigmoid(2z)-1 so all four gates share ONE sigmoid instruction), and
pre-scales conv/pool constants. Device:
  - temporal stats via pool/DVE reductions with per-partition accumulators,
  - 2-layer GraphConv via PE matmuls (adj pre-transposed on host),
  - Conv1D stack via kernel-shifted matmuls,
  - 2-layer LSTM in transposed (units-on-partitions) layout, both layers
    merged per step, single batch group, fused slot-layout cell update.
"""

import os
import numpy as np
from contextlib import ExitStack

import concourse.bass as bass
import concourse.tile as tile
from concourse import bacc, mybir
from concourse.bass_utils import run_bass_kernel_spmd

F32 = mybir.dt.float32
N_CORES = 8
B, H, N, F, P = 64, 168, 512, 8, 24
BL = B // N_CORES          # 8 batches per core
HH = H // 2                # 84
T = H                      # 168 time steps
U = 128                    # LSTM units
GB = BL                    # all 8 batches in one group
NCH = N // 128             # 4 node chunks
NBC = BL * NCH             # 32 (b, nchunk) tiles

_K168 = 1.0 / 168.0
_K84 = 1.0 / 84.0
_KSLOPE = 1.0 / float(168 * (168 * 168 - 1) // 12)  # 1/sum(tc^2)

_CACHE = {}


def _emit_kernel(nc, tc, ctx, dbg=None):
    d = {k: nc.dram_tensor(k, shp, F32, kind="ExternalInput").ap()
         for k, shp in [
             ("x0t", [BL, N, H]), ("seqT", [F, T * BL]), ("adjT", [N, N]),
             ("tc_bc", [128, H]), ("I128", [128, 128]), ("ones_row", [1, 128]),
             ("w1", [7, 32]), ("b1row", [1, 32]), ("w2", [32, 16]),
             ("b2row", [1, 16]),
             ("w1c", [3, N, 4]), ("b1c2", [4, 1]), ("w2ch", [4, 3, 4]),
             ("b2c", [4, 1]),
             ("k1p", [F, 512]), ("rk1p", [U, 512]), ("b1p", [128, 4]),
             ("k2p", [U, 512]), ("rk2p", [U, 512]), ("b2p4", [4, 128]),
             ("sel4", [4, 4 * GB]),
             ("Whead", [16, 4, P]), ("Wlstm", [U, P]), ("b_out_row", [1, P]),
         ]}
    out = nc.dram_tensor("out", [BL, P], F32, kind="ExternalOutput").ap()

    # ---------------- pools (PSUM: 2 + 2 + 2 = 6 banks) ----------------
    consts = ctx.enter_context(tc.tile_pool(name="consts", bufs=1))
    xpool = ctx.enter_context(tc.tile_pool(name="xpool", bufs=3))
    scr = ctx.enter_context(tc.tile_pool(name="scr", bufs=2))
    stats = ctx.enter_context(tc.tile_pool(name="stats", bufs=1))
    gcn = ctx.enter_context(tc.tile_pool(name="gcn", bufs=1))
    lstm = ctx.enter_context(tc.tile_pool(name="lstm", bufs=1))
    ps_zx = ctx.enter_context(tc.tile_pool(name="ps_zx", bufs=2, space="PSUM"))
    ps_a = ctx.enter_context(tc.tile_pool(name="ps_a", bufs=2, space="PSUM"))
    ps_z = ctx.enter_context(tc.tile_pool(name="ps_z", bufs=2, space="PSUM"))

    def load(pool, name, shape=None):
        t = pool.tile(shape or list(d[name].shape), F32, tag=name, name=name)
        nc.sync.dma_start(t[:], d[name][:])
        return t

    # ---------------- resident constants ----------------
    adjT = consts.tile([128, NCH * N], F32, tag="adjT")
    for mc in range(NCH):
        nc.sync.dma_start(adjT[:, mc * N:(mc + 1) * N],
                          d["adjT"][mc * 128:(mc + 1) * 128, :])
    tcb = load(consts, "tc_bc")
    I128 = load(consts, "I128")
    onesr = load(consts, "ones_row")
    w1 = load(consts, "w1")
    b1row = load(consts, "b1row")
    w2 = load(consts, "w2")
    b2row = load(consts, "b2row")
    b1c2 = load(consts, "b1c2")
    w2ch = load(consts, "w2ch")
    b2c = load(consts, "b2c")
    k1p = load(consts, "k1p")
    rk1p = load(consts, "rk1p")
    b1p = load(consts, "b1p")
    k2p = load(consts, "k2p")
    rk2p = load(consts, "rk2p")
    b2p4 = load(consts, "b2p4")
    sel4 = load(consts, "sel4")
    Whead = load(consts, "Whead")
    Wlstm = load(consts, "Wlstm")
    b_out_row = load(consts, "b_out_row")
    seqT = load(consts, "seqT")
    wc1 = consts.tile([128, 3, NCH, 4], F32, tag="wc1sb")
    for dd in range(3):
        for nk in range(NCH):
            nc.sync.dma_start(wc1[:, dd, nk, :],
                              d["w1c"][dd, nk * 128:(nk + 1) * 128, :])

    AL = mybir.AluOpType
    AF = mybir.ActivationFunctionType
    STAGE = int(os.environ.get("KSTAGE", "4"))

    # ================= LSTM x-projection (layer 1), all timesteps =========
    Zx1 = lstm.tile([128, 4, T * BL], F32, tag="Zx1")
    if STAGE < 3:
        nc.vector.memset(Zx1[:], 0.0)
    CW = 448
    nzc = (T * BL + CW - 1) // CW if STAGE >= 3 else 0
    for g in range(4):
        for ci in range(nzc):
            c0, c1 = ci * CW, min((ci + 1) * CW, T * BL)
            pz = ps_zx.tile([128, CW], F32, tag="pzx")
            nc.tensor.matmul(pz[:, :c1 - c0], k1p[:, g * 128:(g + 1) * 128],
                             seqT[:, c0:c1])
            if (g * nzc + ci) % 2 == 0:
                nc.vector.tensor_scalar_add(Zx1[:, g, c0:c1], pz[:, :c1 - c0],
                                            b1p[:, g:g + 1])
            else:
                nc.scalar.activation(Zx1[:, g, c0:c1], pz[:, :c1 - c0],
                                     AF.Identity, bias=b1p[:, g:g + 1])

    # ================= temporal stats ======================================
    S1 = stats.tile([128, NBC], F32, tag="S1")
    S2 = stats.tile([128, NBC], F32, tag="S2")
    S3 = stats.tile([128, NBC], F32, tag="S3")
    S4 = stats.tile([128, NBC], F32, tag="S4")
    S1h = stats.tile([128, NBC], F32, tag="S1h")
    S2h = stats.tile([128, NBC], F32, tag="S2h")
    St = stats.tile([128, NBC], F32, tag="St")
    MEAN = stats.tile([128, NBC], F32, tag="MEAN")

    for b in range(BL):
        for nk in range(NCH):
            col = b * NCH + nk
            xt = xpool.tile([128, H], F32, tag="xt")
            nc.sync.dma_start(xt[:], d["x0t"][b, nk * 128:(nk + 1) * 128, :])
            sc = scr.tile([128, H], F32, tag="csc")
            sc2 = scr.tile([128, H], F32, tag="c2sc")
            sc3 = scr.tile([128, H], F32, tag="c3sc")
            nc.vector.reduce_sum(S1[:, col:col + 1], xt[:],
                                 axis=mybir.AxisListType.X)
            nc.vector.tensor_scalar_mul(MEAN[:, col:col + 1],
                                        S1[:, col:col + 1], _K168)
            nc.vector.tensor_scalar_sub(sc[:], xt[:], MEAN[:, col:col + 1])
            nc.scalar.activation(sc2[:], sc[:], AF.Square,
                                 accum_out=S2[:, col:col + 1])
            nc.vector.scalar_tensor_tensor(sc3[:], sc2[:], 1.0, sc[:],
                                           AL.bypass, AL.mult,
                                           accum_out=S3[:, col:col + 1])
            nc.vector.scalar_tensor_tensor(sc3[:], sc2[:], 1.0, sc2[:],
                                           AL.bypass, AL.mult,
                                           accum_out=S4[:, col:col + 1])
            nc.vector.reduce_sum(S1h[:, col:col + 1], sc[:, HH:],
                                 axis=mybir.AxisListType.X)
            nc.vector.reduce_sum(S2h[:, col:col + 1], sc2[:, HH:],
                                 axis=mybir.AxisListType.X)
            nc.vector.affine_mul_reduce(sc3[:], St[:, col:col + 1], sc[:],
                                        tcb[:], 1.0, 0.0)

    # ---- combine into NF (mean, mean_half, std, std_half, skew, kurt, slope)
    NF = stats.tile([128, 7, NBC], F32, tag="NF")
    w = stats.tile([128, 6, NBC], F32, tag="wrk")
    nc.vector.tensor_copy(NF[:, 0, :], MEAN[:])
    nc.vector.scalar_tensor_tensor(NF[:, 1, :], S1h[:], _K84, MEAN[:],
                                   AL.mult, AL.add)
    nc.vector.tensor_scalar_mul(w[:, 0, :], S2[:], _K168)       # m2
    nc.vector.reciprocal(w[:, 1, :], w[:, 0, :])                # r = 1/m2
    nc.vector.tensor_scalar_mul(w[:, 2, :], S1h[:], _K84)
    nc.gpsimd.tensor_tensor(w[:, 3, :], w[:, 2, :], w[:, 2, :], AL.mult)
    nc.vector.scalar_tensor_tensor(w[:, 3, :], S2h[:], _K84, w[:, 3, :],
                                   AL.mult, AL.subtract)        # var_half
    nc.scalar.activation(NF[:, 2, :], w[:, 0, :], AF.Sqrt)
    nc.scalar.activation(NF[:, 3, :], w[:, 3, :], AF.Sqrt)
    nc.scalar.activation(w[:, 4, :], w[:, 1, :], AF.Sqrt)       # m2^-0.5
    nc.vector.scalar_tensor_tensor(w[:, 5, :], S3[:], _K168, w[:, 1, :],
                                   AL.mult, AL.mult)
    nc.vector.tensor_tensor(NF[:, 4, :], w[:, 5, :], w[:, 4, :], AL.mult)
    nc.vector.scalar_tensor_tensor(w[:, 5, :], S4[:], _K168, w[:, 1, :],
                                   AL.mult, AL.mult)
    nc.gpsimd.tensor_tensor(w[:, 4, :], w[:, 5, :], w[:, 1, :], AL.mult)
    nc.vector.tensor_scalar_add(NF[:, 5, :], w[:, 4, :], -3.0)
    nc.vector.tensor_scalar_mul(NF[:, 6, :], St[:], _KSLOPE)

    if dbg is not None and "nf" in dbg:
        nc.sync.dma_start(dbg["nf"][:], NF[:])

    # ================= GCN =================================================
    if STAGE < 2:
        osb0 = gcn.tile([BL, P], F32, tag="osb0")
        nc.vector.memset(osb0[:], 0.0)
        nc.vector.tensor_tensor(osb0[:, 0:1], NF[0:BL, 0, 0:1], osb0[:, 0:1], AL.add)
        nc.sync.dma_start(out[:], osb0[:])
        return
    NFT = gcn.tile([7, NBC * 128], F32, tag="NFT")
    for q in range(NBC // 4):
        pt = ps_a.tile([7, 512], F32, tag="a")
        for j in range(4):
            nc.tensor.transpose(pt[:, j * 128:(j + 1) * 128],
                                NF[:, :, q * 4 + j], I128[:])
        nc.vector.tensor_copy(NFT[:, q * 512:(q + 1) * 512], pt[:])

    T1 = gcn.tile([128, NBC, 32], F32, tag="T1")
    for bc in range(NBC):
        pt = ps_a.tile([128, 32], F32, tag="a")
        nc.tensor.matmul(pt[:], NFT[:, bc * 128:(bc + 1) * 128], w1[:])
        nc.vector.tensor_copy(T1[:, bc, :], pt[:])

    H1 = gcn.tile([128, NBC, 32], F32, tag="H1")
    for b in range(BL):
        for nk in range(NCH):
            ph = ps_a.tile([128, 32], F32, tag="a")
            for mc in range(NCH):
                nc.tensor.matmul(ph[:], adjT[:, mc * N + nk * 128:
                                              mc * N + (nk + 1) * 128],
                                 T1[:, b * NCH + mc, :],
                                 start=(mc == 0), stop=False)
            nc.tensor.matmul(ph[:], onesr[:1, :], b1row[:], start=False,
                             stop=True)
            nc.vector.tensor_scalar_max(H1[:, b * NCH + nk, :], ph[:], 0.0)

    H1T = gcn.tile([32, NBC * 128], F32, tag="H1T")
    for q in range(NBC // 4):
        pt = ps_a.tile([32, 512], F32, tag="a")
        for j in range(4):
            nc.tensor.transpose(pt[:, j * 128:(j + 1) * 128],
                                H1[:, q * 4 + j, :], I128[:])
        nc.vector.tensor_copy(H1T[:, q * 512:(q + 1) * 512], pt[:])

    T2 = gcn.tile([128, NBC, 16], F32, tag="T2")
    for bc in range(NBC):
        pt = ps_a.tile([128, 16], F32, tag="a")
        nc.tensor.matmul(pt[:], H1T[:, bc * 128:(bc + 1) * 128], w2[:])
        nc.vector.tensor_copy(T2[:, bc, :], pt[:])

    G = gcn.tile([128, NBC, 16], F32, tag="G")
    for b in range(BL):
        for nk in range(NCH):
            ph = ps_a.tile([128, 16], F32, tag="a")
            for mc in range(NCH):
                nc.tensor.matmul(ph[:], adjT[:, mc * N + nk * 128:
                                              mc * N + (nk + 1) * 128],
                                 T2[:, b * NCH + mc, :],
                                 start=(mc == 0), stop=False)
            nc.tensor.matmul(ph[:], onesr[:1, :], b2row[:], start=False,
                             stop=True)
            nc.vector.tensor_scalar_max(G[:, b * NCH + nk, :], ph[:], 0.0)

    if dbg is not None and "g" in dbg:
        nc.sync.dma_start(dbg["g"][:], G[:])

    # ================= Conv1D head ========================================
    # c1[o, 16b+l] = sum_d sum_n g[b, n, l+d-1] * w1c[d, n, o]
    pc1 = ps_a.tile([4, 16 * BL], F32, tag="a")
    for b in range(BL):
        first = True
        for dd in (1, 0, 2):  # full-width shift first (start=True coverage)
            lo, hi = max(0, 1 - dd), min(16, 17 - dd)
            for nk in range(NCH):
                nc.tensor.matmul(
                    pc1[:, 16 * b + lo:16 * b + hi],
                    wc1[:, dd, nk, :],
                    G[:, b * NCH + nk, lo + dd - 1:hi + dd - 1],
                    start=first, stop=(dd == 2 and nk == NCH - 1))
                first = False
    c1sb = gcn.tile([4, 16 * BL], F32, tag="c1sb")
    nc.vector.tensor_copy(c1sb[:], pc1[:])
    # p' = c1e + c1o + 2*b_conv1  (scale 0.5 folded into w2ch/Whead)
    GH = gcn.tile([4, 16 * BL], F32, tag="GH")   # per b: [c2(8) | p'(8)]
    pv = GH[:].rearrange("p (b h l) -> p b h l", b=BL, h=2)
    c1v = c1sb[:].rearrange("p (b l e) -> p b l e", b=BL, e=2)
    nc.vector.scalar_tensor_tensor(pv[:, :, 1, :], c1v[:, :, :, 0], b1c2[:],
                                   c1v[:, :, :, 1], AL.add, AL.add)
    # c2 = conv2(p') + b_conv2
    pc2 = ps_a.tile([4, 8 * BL], F32, tag="a")
    for b in range(BL):
        first = True
        for dd in (1, 0, 2):
            lo, hi = max(0, 1 - dd), min(8, 9 - dd)
            nc.tensor.matmul(pc2[:, 8 * b + lo:8 * b + hi],
                             w2ch[:, dd, :],
                             pv[:, b, 1, lo + dd - 1:hi + dd - 1],
                             start=first, stop=(dd == 2))
            first = False
    pc2v = pc2[:].rearrange("p (b l) -> p b l", b=BL)
    nc.vector.tensor_scalar_add(pv[:, :, 0, :], pc2v[:], b2c[:])
    # transpose per b: (4, 16) -> (16, 4); featT cols = 4b + o
    pft = ps_a.tile([16, 4 * BL], F32, tag="a")
    for b in range(BL):
        nc.tensor.transpose(pft[:, 4 * b:4 * b + 4],
                            GH[:, 16 * b:16 * (b + 1)], I128[:4, :4])
    featT = gcn.tile([16, 4 * BL], F32, tag="featT")
    nc.vector.tensor_copy(featT[:], pft[:])

    # ================= LSTM recurrence ====================================
    # gt slots: 0=i 1=f 2=o 3=g' 4=c 5=th; fused update:
    #   [u|fc] = [i|f] * [g'|c];  c = u + fc;  th = tanh(c);  h = o * th
    gt = lstm.tile([128, 2, 6, GB], F32, tag="gt", name="gt")
    uv = lstm.tile([128, 2, 2, GB], F32, tag="uv", name="uv")
    hh = lstm.tile([128, 2, GB], F32, tag="hh", name="hh")
    nc.vector.memset(gt[:, :, 4, :], 0.0)   # c = 0
    nc.vector.memset(hh[:], 0.0)

    Zx1v = Zx1[:].rearrange("p g (t b) -> p g t b", b=BL)

    TSTEPS = (T + 1) if STAGE >= 4 else 0
    for t in range(TSTEPS):
        pz = ps_z.tile([128, 2, 4, GB], F32, tag="pz")
        do1, do2 = t < T, t > 0
        if do1:
            nc.tensor.matmul(pz[:, 0, :, :], I128[:],
                             Zx1v[:, :, t, :],
                             start=True, stop=(t == 0))
            if t > 0:
                for g in range(4):
                    nc.tensor.matmul(pz[:, 0, g, :],
                                     rk1p[:, g * 128:(g + 1) * 128],
                                     hh[:, 0, :],
                                     start=False, stop=(g == 3))
        if do2:
            nc.tensor.matmul(pz[:, 1, :, :], b2p4[:], sel4[:],
                             start=True, stop=False)
            for g in range(4):
                nc.tensor.matmul(pz[:, 1, g, :],
                                 k2p[:, g * 128:(g + 1) * 128],
                                 hh[:, 0, :], start=False,
                                 stop=(t == 1 and g == 3))
            if t > 1:
                for g in range(4):
                    nc.tensor.matmul(pz[:, 1, g, :],
                                     rk2p[:, g * 128:(g + 1) * 128],
                                     hh[:, 1, :],
                                     start=False, stop=(g == 3))

        l0, l1 = (0 if do1 else 1), (2 if do2 else 1)
        nc.scalar.activation(gt[:, l0:l1, 0:4, :], pz[:, l0:l1, :, :],
                             AF.Sigmoid)
        nc.vector.tensor_scalar(gt[:, l0:l1, 3, :], gt[:, l0:l1, 3, :],
                                2.0, -1.0, AL.mult, AL.add)
        nc.vector.tensor_tensor(uv[:, l0:l1, :, :], gt[:, l0:l1, 0:2, :],
                                gt[:, l0:l1, 3:5, :], AL.mult)
        nc.vector.tensor_tensor(gt[:, l0:l1, 4, :], uv[:, l0:l1, 0, :],
                                uv[:, l0:l1, 1, :], AL.add)
        nc.scalar.activation(gt[:, l0:l1, 5, :], gt[:, l0:l1, 4, :],
                             AF.Tanh)
        nc.vector.tensor_tensor(hh[:, l0:l1, :], gt[:, l0:l1, 2, :],
                                gt[:, l0:l1, 5, :], AL.mult)

    # ================= output head ========================================
    po = ps_a.tile([BL, P], F32, tag="a")
    nc.tensor.matmul(po[:], onesr[:1, :BL], b_out_row[:], start=True,
                     stop=False)
    fv = featT[:].rearrange("p (b o) -> p b o", o=4)
    for o in range(4):
        nc.tensor.matmul(po[:], fv[:, :, o], Whead[:, o, :], start=False,
                         stop=False)
    nc.tensor.matmul(po[:], hh[:, 1, :], Wlstm[:], start=False, stop=True)
    osb = gcn.tile([BL, P], F32, tag="osb")
    nc.vector.tensor_copy(osb[:], po[:])
    nc.sync.dma_start(out[:], osb[:])


def _build(dbg_names=()):
    key = tuple(sorted(dbg_names))
    if key in _CACHE:
        return _CACHE[key]
    nc = bacc.Bacc("TRN2", target_bir_lowering=False, debug=False,
                   num_devices=N_CORES)
    with tile.TileContext(nc) as tc:
        with ExitStack() as ctx:
            dbg = {}
            if "nf" in key:
                dbg["nf"] = nc.dram_tensor("dbg_nf", [128, 7, NBC], F32,
                                           kind="ExternalOutput").ap()
            if "g" in key:
                dbg["g"] = nc.dram_tensor("dbg_g", [128, NBC, 16], F32,
                                          kind="ExternalOutput").ap()
            _emit_kernel(nc, tc, ctx, dbg=dbg or None)
    nc.compile()
    _CACHE[key] = nc
    return nc


def _prep(inputs):
    x0 = np.ascontiguousarray(inputs["inputs"][..., 0])          # (B, H, N)
    x0t = np.ascontiguousarray(x0.transpose(0, 2, 1))            # (B, N, H)
    seq = inputs["inputs"][:, :, 0, :]                           # (B, H, F)
    adjT = np.ascontiguousarray(inputs["adj"].T)
    tc_vec = (np.arange(H, dtype=np.float32) - (H - 1) / 2.0)
    tc_bc = np.broadcast_to(tc_vec, (128, H)).copy()
    I128 = np.eye(128, dtype=np.float32)
    ones_row = np.ones((1, 128), np.float32)

    perm = np.concatenate([np.arange(0, 128), np.arange(128, 256),
                           np.arange(384, 512), np.arange(256, 384)])
    # double the g-gate (cols 384:512 after perm): tanh(z) = 2*sig(2z)-1
    gsc = np.ones((512,), np.float32)
    gsc[384:512] = 2.0
    k1p = inputs["k_lstm1"][:, perm] * gsc
    rk1p = inputs["rk_lstm1"][:, perm] * gsc
    b1p = (inputs["b_lstm1"][perm] * gsc).reshape(4, 128).T
    k2p = inputs["k_lstm2"][:, perm] * gsc
    rk2p = inputs["rk_lstm2"][:, perm] * gsc
    b2p4 = (inputs["b_lstm2"][perm] * gsc).reshape(4, 128)
    sel4 = np.zeros((4, 4 * GB), np.float32)
    for g in range(4):
        sel4[g, g * GB:(g + 1) * GB] = 1.0

    w_out = inputs["w_out"]
    Whead = np.zeros((16, 4, P), np.float32)
    for o in range(4):
        for l in range(8):
            Whead[l, o, :] = w_out[o * 8 + l, :]                 # c2 rows
            Whead[8 + l, o, :] = 0.5 * w_out[32 + o * 8 + l, :]  # p rows
    Wlstm = w_out[64:192, :]

    com = {
        "adjT": adjT, "tc_bc": tc_bc, "I128": I128, "ones_row": ones_row,
        "w1": inputs["w_gcn1"], "b1row": inputs["b_gcn1"][None, :],
        "w2": inputs["w_gcn2"], "b2row": inputs["b_gcn2"][None, :],
        "w1c": inputs["w_conv1"], "b1c2": 2.0 * inputs["b_conv1"][:, None],
        "w2ch": 0.5 * np.asarray(inputs["w_conv2"]).transpose(1, 0, 2),
        "b2c": inputs["b_conv2"][:, None],
        "k1p": k1p, "rk1p": rk1p, "b1p": b1p, "k2p": k2p, "rk2p": rk2p,
        "b2p4": b2p4, "sel4": sel4, "Whead": Whead, "Wlstm": Wlstm,
        "b_out_row": inputs["b_out"][None, :],
    }
    com = {k: np.ascontiguousarray(v, dtype=np.float32)
           for k, v in com.items()}

    in_maps = []
    for c in range(N_CORES):
        bs = slice(c * BL, (c + 1) * BL)
        m = dict(com)
        m["x0t"] = np.ascontiguousarray(x0t[bs])
        m["seqT"] = np.ascontiguousarray(
            np.asarray(seq[bs]).transpose(2, 1, 0).reshape(F, T * BL))
        in_maps.append(m)
    return in_maps


def kernel(**inputs):
    nc = _build()
    in_maps = _prep(inputs)
    res = run_bass_kernel_spmd(nc, in_maps, list(range(N_CORES)))
    return np.concatenate([res.results[c]["out"] for c in range(N_CORES)],
                          axis=0)
